# revision 2
# baseline (speedup 1.0000x reference)
"""Trainium2 Bass kernel v3 for nn_Decoder.

v2 -> v3: inputs shrink from per-image splat planes (11MB/core) to raw
coords/values (1.6MB, replicated) + 32B/image rotation rows; all per-point
planes (px, py, y0, a0, a1, x0, b0, b1, pxn) are derived ON DEVICE with
~18 wide DVE ops per image (~9us).  This removes most per-call input
shipping AND most host prep.

Splat per chunk (128 points) unchanged from v2:
  DVE: t1=(io==y0)*a0, t2=(io-1==y0)*a1
  X: (MIX-1)/MIX chunks: hat on ACT (Abs; Relu(1-.)); 1/MIX chunks:
     one-hots on DVE + 4 extra matmuls.
  PE: img_ps[h] += t_i[:,h*128:]^T @ x_j

On-device floor trick: round(z-0.499) = ((z - 0.499) + 1.5*2^23) - 1.5*2^23
(two ts ops, RNE quantizes to integer at ulp=1).
"""

import os

import ml_dtypes
import numpy as np

import concourse.bass as bass
import concourse.mybir as mybir
import concourse.tile as tile_mod
from concourse.bass_utils import run_bass_kernel_spmd
from concourse.tile import TileContext
from concourse.vector_clock import ScopedClock

B = 32
N = 100000
XS = 256
KSIZE = 5
N_CORES = 8
IMG_PER_CORE = B // N_CORES
NP = ((N + 127) // 128) * 128  # 100096
CH = NP // 128  # 782
F32 = mybir.dt.float32
F32R = mybir.dt.float32r
BF16 = mybir.dt.bfloat16
AF = mybir.ActivationFunctionType
ALU = mybir.AluOpType
NPBF16 = ml_dtypes.bfloat16

STAGE_DT = F32R
STAGE_NP = np.float32
MIX = int(os.environ.get("BASS3_MIX", "4"))
BUFS = int(os.environ.get("BASS3_BUFS", "8"))
BIG = 12582912.0  # 1.5 * 2**23

# ---------------------------------------------------------------------------
_PATCHED = False


def _patch_tile_drain():
    global _PATCHED
    if _PATCHED:
        return
    _PATCHED = True

    def _drain_and_barrier(self, tick_clock, wait_clock):
        probe = self.nc.sync.nop(nofuse=True, hint="drain_wait_probe")
        wait_clock.add_sem_waits(
            probe.ins, ScopedClock({None: tick_clock.global_clock})
        )
        si = probe.ins.sync_info
        waits = list(si.on_wait) if si is not None else []
        probe.ins.sync_info = mybir.SyncInfo(on_wait=waits[:1], on_update=[])
        for w in waits[1:]:
            n = self.nc.sync.nop(nofuse=True, hint="drain_wait_extra")
            n.ins.sync_info = mybir.SyncInfo(on_wait=[w], on_update=[])
        self.nc.sync.drain()
        self.nc.all_engine_barrier()
        assert self.sems is not None
        popped = self.nc._tile_sem_poison_stack.pop()
        assert popped is self._sem_poison
        self.nc.clear_and_free_semaphores(list(self.sems.allocated().values()))
        self.nc.all_engine_barrier()

    tile_mod.TileContext._drain_and_barrier = _drain_and_barrier


def _split_excess_waits(nc):
    n = 0
    for fn in nc.m.functions:
        for bb in fn.blocks:
            il = bb.instructions
            out = []
            changed = False
            for ins in il:
                si = ins.sync_info
                if si is not None and len(si.on_wait) > 1:
                    waits = list(si.on_wait)
                    for w in waits[:-1]:
                        n += 1
                        nop = mybir.InstNoOp(
                            name=f"I-waitsplit-{n}", ins=[], outs=[]
                        )
                        nop.engine = ins.engine
                        nop.sync_info = mybir.SyncInfo(
                            on_wait=[w], on_update=[]
                        )
                        nc.register_instruction(nop)
                        out.append(nop)
                    ins.sync_info = mybir.SyncInfo(
                        on_wait=[waits[-1]], on_update=list(si.on_update)
                    )
                    changed = True
                out.append(ins)
            if changed:
                bb.instructions = out


# ---------------------------------------------------------------------------
# Host-side math


def _rot6d(alignment):
    a1, a2 = alignment[:, :3], alignment[:, 3:]
    b1 = a1 / (np.linalg.norm(a1, axis=-1, keepdims=True) + 1e-8)
    a2p = a2 - np.sum(b1 * a2, axis=-1, keepdims=True) * b1
    b2 = a2p / (np.linalg.norm(a2p, axis=-1, keepdims=True) + 1e-8)
    b3 = np.cross(b1, b2)
    return np.stack([b1, b2, b3], axis=1)


def _conv_matrix(g1, n):
    m = np.zeros((n, n), np.float64)
    for i in range(n):
        for u in range(KSIZE):
            j = i + u - KSIZE // 2
            if 0 <= j < n:
                m[i, j] += g1[u]
    return m


DFT_NAMES = [
    "wgy_t_r", "wgy_t_i",
    "wgx_t_r", "wgx_t_i", "wgx_t_in",
    "wit_r", "wit_i", "wit_in",
]


def _dft_consts(gauss_kernel):
    u, s, vt = np.linalg.svd(gauss_kernel.astype(np.float64))
    gy = np.sqrt(s[0]) * u[:, 0]
    gx = np.sqrt(s[0]) * vt[0, :]
    if gy[KSIZE // 2] < 0:
        gy, gx = -gy, -gx
    k = np.arange(XS)
    w = np.exp(-2j * np.pi * np.outer(k, k) / XS)
    winv = np.conj(w) / XS
    wgy_t = (w @ _conv_matrix(gy, XS)).T
    wgx_t = (w @ _conv_matrix(gx, XS)).T
    wit = winv.T
    consts = {
        "wgy_t_r": np.real(wgy_t),
        "wgy_t_i": np.imag(wgy_t),
        "wgx_t_r": np.real(wgx_t),
        "wgx_t_i": np.imag(wgx_t),
        "wgx_t_in": -np.imag(wgx_t),
        "wit_r": np.real(wit),
        "wit_i": np.imag(wit),
        "wit_in": -np.imag(wit),
    }
    return {
        name: np.ascontiguousarray(m.reshape(2, 128, XS).astype(STAGE_NP))
        for name, m in consts.items()
    }


# ---------------------------------------------------------------------------
# Device program

_PROGRAM = None


def build_program(img_per_core=IMG_PER_CORE, n_chunks=CH):
    _patch_tile_drain()
    nc = bass.Bass()

    # coords as plane layout [128, 3, CH] (cx, cy, cz), values [128, CH]
    cpl = nc.declare_dram_parameter("cpl", [128, 3, CH], F32, isOutput=False)
    vpl = nc.declare_dram_parameter("vpl", [128, CH], F32, isOutput=False)
    # per-image projection rows, broadcast across partitions:
    # rxy[img, 128, 8] = (r0x,r1x,r2x,t0x, r0y,r1y,r2y,t0y)
    rxy = nc.declare_dram_parameter("rxy", [img_per_core, 128, 8], F32,
                                    isOutput=False)
    iota16 = nc.declare_dram_parameter("iota16", [2, 128, XS], BF16,
                                       isOutput=False)
    iota32 = nc.declare_dram_parameter("iota32", [128, XS], F32,
                                       isOutput=False)
    ctf = nc.declare_dram_parameter(
        "ctf", [img_per_core, 2, 128, XS], F32, isOutput=False
    )
    dft = {
        name: nc.declare_dram_parameter(name, [2, 128, XS], STAGE_DT,
                                        isOutput=False)
        for name in DFT_NAMES
    }
    out = nc.declare_dram_parameter(
        "out", [img_per_core, XS, XS], F32, isOutput=True
    )

    with TileContext(nc) as tc:
        with (
            tc.tile_pool(name="const", bufs=1) as cpool,
            tc.tile_pool(name="planes", bufs=2) as ppool,
            tc.tile_pool(name="deriv", bufs=2) as dpool,
            tc.tile_pool(name="build", bufs=BUFS) as bpool,
            tc.tile_pool(name="stage", bufs=2) as spool,
            tc.tile_pool(name="psum", bufs=4, space="PSUM") as qpool,
        ):
            io16 = cpool.tile([128, XS], BF16, tag="io16", name="io16")
            nc.sync.dma_start(out=io16[:], in_=iota16[0])
            io16m1 = cpool.tile([128, XS], BF16, tag="io16m1", name="io16m1")
            nc.sync.dma_start(out=io16m1[:], in_=iota16[1])
            io32 = cpool.tile([128, XS], F32, tag="io32", name="io32")
            nc.sync.dma_start(out=io32[:], in_=iota32[:])
            ct = cpool.tile([128, 3, CH], F32, tag="cpl", name="ct")
            nc.sync.dma_start(out=ct[:], in_=cpl[:])
            vt = cpool.tile([128, CH], F32, tag="vpl", name="vt")
            nc.sync.dma_start(out=vt[:], in_=vpl[:])
            dft_t = {}
            for name in DFT_NAMES:
                for kc in range(2):
                    t = cpool.tile([128, XS], STAGE_DT, tag=f"{name}{kc}",
                                   name=f"c_{name}{kc}")
                    nc.sync.dma_start(out=t[:], in_=dft[name][kc])
                    dft_t[name, kc] = t

            for b in range(img_per_core):
                rx = ppool.tile([128, 8], F32, tag="rxy", name="rx")
                nc.sync.dma_start(out=rx[:], in_=rxy[b])
                ctf_t = [ppool.tile([128, XS], F32, tag=f"ctf{h}",
                                    name=f"ctf_t{h}") for h in range(2)]
                for h in range(2):
                    nc.sync.dma_start(out=ctf_t[h][:], in_=ctf[b, h])

                # ---- derive per-point planes on device ----
                def proj(base):
                    # (r0*cx + r1*cy + r2*cz + t) clipped to [0, 255]
                    m3 = dpool.tile([128, CH], F32, tag=f"m3{base}", name="m3")
                    nc.vector.tensor_scalar(
                        m3[:], ct[:, 2], rx[:, base + 2 : base + 3],
                        rx[:, base + 3 : base + 4], ALU.mult, ALU.add)
                    m2 = dpool.tile([128, CH], F32, tag=f"m2{base}", name="m2")
                    nc.vector.scalar_tensor_tensor(
                        m2[:], ct[:, 1], rx[:, base + 1 : base + 2], m3[:],
                        ALU.mult, ALU.add)
                    m1 = dpool.tile([128, CH], F32, tag=f"m1{base}", name="m1")
                    nc.vector.scalar_tensor_tensor(
                        m1[:], ct[:, 0], rx[:, base : base + 1], m2[:],
                        ALU.mult, ALU.add)
                    p = dpool.tile([128, CH], F32, tag=f"p{base}", name="p")
                    nc.vector.tensor_scalar(
                        p[:], m1[:], 0.0, float(XS - 1), ALU.max, ALU.min)
                    return p

                px = proj(0)
                py = proj(4)

                def floor_frac(p, tag):
                    r = dpool.tile([128, CH], F32, tag=f"r{tag}", name="r")
                    nc.vector.tensor_scalar(
                        r[:], p[:], 0.499, BIG, ALU.subtract, ALU.add)
                    i0 = ppool.tile([128, CH], F32, tag=f"i0{tag}", name="i0")
                    nc.vector.tensor_scalar(i0[:], r[:], BIG, None,
                                            ALU.subtract)
                    f = ppool.tile([128, CH], F32, tag=f"f{tag}", name="f")
                    nc.vector.tensor_sub(f[:], p[:], i0[:])
                    return i0, f

                y0, fy = floor_frac(py, "y")
                x0, fx = floor_frac(px, "x")
                # a1 = fy*v ; a0 = v - a1
                a1p = ppool.tile([128, CH], F32, tag="a1", name="a1")
                nc.vector.tensor_mul(a1p[:], fy[:], vt[:])
                a0p = ppool.tile([128, CH], F32, tag="a0", name="a0")
                nc.vector.scalar_tensor_tensor(
                    a0p[:], a1p[:], -1.0, vt[:], ALU.mult, ALU.add)
                # b0 = 1 - fx ; b1 = fx
                b0p = ppool.tile([128, CH], F32, tag="b0", name="b0")
                nc.vector.tensor_scalar(
                    b0p[:], fx[:], -1.0, 1.0, ALU.mult, ALU.add)
                pxn = ppool.tile([128, CH], F32, tag="pxn", name="pxn")
                nc.vector.tensor_scalar(pxn[:], px[:], -1.0, None, ALU.mult)

                planes = {"y0": y0, "a0": a0p, "a1": a1p, "pxn": pxn,
                          "x0": x0, "b0": b0p, "b1": fx}

                def pcol(plane, c):
                    return planes[plane][:, c : c + 1]

                # ---- splat ----
                img_ps = [
                    qpool.tile([128, XS], F32, tag="psA", name=f"img_ps{h}")
                    for h in range(2)
                ]
                for c in range(n_chunks):
                    t1 = bpool.tile([128, XS], BF16, tag="t1", name="t1")
                    t2 = bpool.tile([128, XS], BF16, tag="t2", name="t2")
                    nc.vector.tensor_scalar(
                        t1[:], io16[:], pcol("y0", c), pcol("a0", c),
                        ALU.is_equal, ALU.mult,
                    )
                    nc.vector.tensor_scalar(
                        t2[:], io16m1[:], pcol("y0", c), pcol("a1", c),
                        ALU.is_equal, ALU.mult,
                    )
                    if c % MIX == MIX - 1:
                        x1 = bpool.tile([128, XS], BF16, tag="x1", name="x1")
                        x2 = bpool.tile([128, XS], BF16, tag="x2", name="x2")
                        nc.vector.tensor_scalar(
                            x1[:], io16[:], pcol("x0", c), pcol("b0", c),
                            ALU.is_equal, ALU.mult,
                        )
                        nc.vector.tensor_scalar(
                            x2[:], io16m1[:], pcol("x0", c), pcol("b1", c),
                            ALU.is_equal, ALU.mult,
                        )
                        xts = [x1, x2]
                    else:
                        xa = bpool.tile([128, XS], BF16, tag="xa", name="xa")
                        nc.scalar.activation(
                            xa[:], io32[:], AF.Abs,
                            bias=pcol("pxn", c), scale=1.0,
                        )
                        xh = bpool.tile([128, XS], BF16, tag="xh", name="xh")
                        nc.scalar.activation(
                            xh[:], xa[:], AF.Relu, bias=1.0, scale=-1.0,
                        )
                        xts = [xh]
                    for h in range(2):
                        for ti, tt in enumerate((t1, t2)):
                            for xi, xt in enumerate(xts):
                                first = (c == 0 and ti == 0 and xi == 0)
                                last = (
                                    c == n_chunks - 1
                                    and ti == 1
                                    and xi == len(xts) - 1
                                )
                                nc.tensor.matmul(
                                    img_ps[h][:],
                                    tt[:, h * 128 : (h + 1) * 128],
                                    xt[:],
                                    start=first,
                                    stop=last,
                                )

                img_sb = [
                    spool.tile([128, XS], STAGE_DT, tag=f"isb{h}",
                               name=f"isb{h}") for h in range(2)
                ]
                for h in range(2):
                    nc.vector.tensor_copy(img_sb[h][:], img_ps[h][:])

                # ---- DFT chain ----
                def product(terms, tag, ps_tag, mult_by=None):
                    res = []
                    for ho in range(2):
                        ps = qpool.tile([128, XS], F32, tag=ps_tag,
                                        name=f"ps_{tag}{ho}")
                        nmm = 2 * len(terms)
                        i = 0
                        for lhs_tiles, rhs_name in terms:
                            for kc in range(2):
                                nc.tensor.matmul(
                                    ps[:],
                                    lhs_tiles[kc][
                                        :, ho * 128 : (ho + 1) * 128
                                    ],
                                    dft_t[rhs_name, kc][:],
                                    start=(i == 0),
                                    stop=(i == nmm - 1),
                                )
                                i += 1
                        sb = spool.tile([128, XS], STAGE_DT,
                                        tag=f"sb{tag}{ho}",
                                        name=f"sb{tag}{ho}")
                        if mult_by is not None:
                            nc.vector.tensor_mul(sb[:], ps[:],
                                                 mult_by[ho][:])
                        else:
                            nc.vector.tensor_copy(sb[:], ps[:])
                        res.append(sb)
                    return res

                ar = product([(img_sb, "wgy_t_r")], "ar", "psB")
                ai = product([(img_sb, "wgy_t_i")], "ai", "psB")
                fr = product(
                    [(ar, "wgx_t_r"), (ai, "wgx_t_in")], "fr", "psA",
                    mult_by=ctf_t,
                )
                fi = product(
                    [(ar, "wgx_t_i"), (ai, "wgx_t_r")], "fi", "psA",
                    mult_by=ctf_t,
                )
                br = product([(fr, "wit_r"), (fi, "wit_in")], "br", "psB")
                bi = product([(fr, "wit_i"), (fi, "wit_r")], "bi", "psB")
                for ho in range(2):
                    ps = qpool.tile([128, XS], F32, tag="psA",
                                    name=f"ps_o{ho}")
                    i = 0
                    for lhs_tiles, rhs_name in [(br, "wit_r"), (bi, "wit_in")]:
                        for kc in range(2):
                            nc.tensor.matmul(
                                ps[:],
                                lhs_tiles[kc][:, ho * 128 : (ho + 1) * 128],
                                dft_t[rhs_name, kc][:],
                                start=(i == 0),
                                stop=(i == 3),
                            )
                            i += 1
                    osb = spool.tile([128, XS], F32, tag=f"osb{ho}",
                                     name=f"osb{ho}")
                    nc.vector.tensor_copy(osb[:], ps[:])
                    nc.sync.dma_start(
                        out=out[b, ho * 128 : (ho + 1) * 128, :], in_=osb[:]
                    )
    _split_excess_waits(nc)
    return nc


def _prep_host(alignment, shifts, coords, values, gauss_kernel, ctf,
               img_per_core=IMG_PER_CORE):
    rot = _rot6d(alignment.astype(np.float64))
    nb = rot.shape[0]
    half = XS // 2

    # coords/values in padded plane layout [128, ., CH]
    cpad = np.zeros((3, NP), np.float32)
    cpad[:, :N] = coords.astype(np.float32).T
    # pad coords far outside so derived px,py clip to border with v=0
    cpad[:, N:] = 0.0
    cpl = np.ascontiguousarray(
        cpad.reshape(3, CH, 128).transpose(2, 0, 1)).astype(np.float32)
    vpad = np.zeros(NP, np.float32)
    vpad[:N] = values.astype(np.float32)
    vpl = np.ascontiguousarray(
        vpad.reshape(CH, 128).T).astype(np.float32)

    # per-image projection rows: px = r0*cx+r1*cy+r2*cz + (sx+half)
    rxy = np.empty((nb, 128, 8), np.float32)
    for b in range(nb):
        row = np.concatenate([
            rot[b, 0, :], [shifts[b, 0] + half],
            rot[b, 1, :], [shifts[b, 1] + half],
        ]).astype(np.float32)
        rxy[b] = row[None, :]

    iota = np.arange(XS, dtype=np.float64)
    iota16 = np.ascontiguousarray(
        np.stack([
            np.broadcast_to(iota, (128, XS)),
            np.broadcast_to(iota - 1.0, (128, XS)),
        ]).astype(NPBF16)
    )
    iota32 = np.ascontiguousarray(
        np.broadcast_to(iota, (128, XS)).astype(np.float32)
    )
    consts = _dft_consts(gauss_kernel)
    cs = np.fft.ifftshift(ctf.astype(np.float32), axes=(-2, -1))
    cs = np.ascontiguousarray(cs.reshape(nb, 2, 128, XS))

    n_cores = nb // img_per_core
    in_maps = []
    for core in range(n_cores):
        sl = slice(core * img_per_core, (core + 1) * img_per_core)
        m = {
            "cpl": cpl, "vpl": vpl,
            "rxy": np.ascontiguousarray(rxy[sl]),
            "iota16": iota16, "iota32": iota32,
            "ctf": np.ascontiguousarray(cs[sl]),
        }
        m.update(consts)
        in_maps.append(m)
    return in_maps


def kernel(alignment, shifts, coords, values, gauss_kernel, ctf):
    global _PROGRAM
    if _PROGRAM is None:
        _PROGRAM = build_program()
    in_maps = _prep_host(
        np.asarray(alignment), np.asarray(shifts), np.asarray(coords),
        np.asarray(values), np.asarray(gauss_kernel), np.asarray(ctf),
    )
    res = run_bass_kernel_spmd(_PROGRAM, in_maps, list(range(N_CORES)))
    return np.concatenate([r["out"] for r in res.results], axis=0)
